# revision 30
# baseline (speedup 1.0000x reference)
"""Trainium2 Bass kernel for nn_DispersiveLoss (B=2048, D=16*768=12288, 8 cores).

Strategy (circulant block decomposition, uniform SPMD, single launch):
  x (2048, 12288) -> 16 row-blocks of 128. Core c owns m-blocks {2c, 2c+1}
  and computes two Gram strips G[m, m..m+8 (mod 16)] (width 9 blocks = 1152)
  in fp8 DoubleRow (D on partitions, 48 two-chunk k-steps, PSUM fp32).
  Every unordered block pair lands exactly once (circular distance 1..7),
  diagonal blocks are masked to the upper triangle, distance-8 blocks are
  computed twice and weighted 0.5.

  Row norms sq are computed on the HOST (O(B*D) prep, like the transpose).
  The per-column correction v_j = (mean(sq) - sq_j)/2 is folded into PSUM
  by K=1 bf16 matmuls (one per PSUM tile); the per-row term rides the ACT
  bias in exact f32. PSUM thus holds P = g + v_j; ACT produces
  exp(2*SS*P + bias_i) and DVE produces sum(P), sum(P^2) per region with
  accum_out row-sums. Host reconstructs S1/S2 exactly in f64 (it knows the
  bf16 quantization residuals), so only fp8 Gram noise remains.

  The union columns stream in two phases (A: union cols 0:640 incl. both
  lhs blocks, kept resident; B: cols 640:1280, ring-buffered) so phase A's
  post-processing overlaps phase B's matmuls and the serial tail is only
  phase B's post (~3us).
"""

import os

import numpy as np
import ml_dtypes

import concourse.bass as bass
import concourse.mybir as mybir
import concourse.tile as tile
from concourse import bacc
from concourse.bass_utils import run_bass_kernel_spmd

NC_N = 8
B, D = 2048, 12288
BLK = 128
UNION = 1280
KCH = 96  # k-chunks of 128
TAU = 0.5
CC = float(2 * D)
SS = 1.0 / (D * TAU)
S2E = 2.0 * SS
F32 = mybir.dt.float32
BF16 = mybir.dt.bfloat16
FP8 = mybir.dt.float8e4
NP_FP8 = ml_dtypes.float8_e4m3
NP_BF16 = ml_dtypes.bfloat16
LN_HALF = float(np.log(0.5))
N_PAIRS = B * (B - 1) // 2

KERNEL_EXEC_NS = []  # filled when KERNEL_TRACE is set (test harness only)

_cache = {}


def _trace_enabled():
    return bool(os.environ.get("KERNEL_TRACE"))


def _build_kernel():
    nc = bacc.Bacc("TRN2", target_bir_lowering=False, debug=False, num_devices=NC_N)
    xA = nc.dram_tensor("xA", [BLK, KCH, 640], FP8, kind="ExternalInput")
    xB = nc.dram_tensor("xB", [BLK, KCH, 640], FP8, kind="ExternalInput")
    auxf = nc.dram_tensor("auxf", [BLK, 132], F32, kind="ExternalInput")
    auxb = nc.dram_tensor("auxb", [1, 1408], BF16, kind="ExternalInput")
    # out cols: 0:6 diag acc [E_d0, E_d1, S1_d0, S1_d1, S2_d0, S2_d1],
    # 6:48 P-stats (7 regions x 6 bn_stats), 48:90 E-stats (7 x 6)
    out_stats = nc.dram_tensor("out_stats", [BLK, 92], F32, kind="ExternalOutput")

    MULT = mybir.AluOpType.mult
    ADD = mybir.AluOpType.add
    EXP = mybir.ActivationFunctionType.Exp
    DR = mybir.MatmulPerfMode.DoubleRow

    # A-phase DMA batches (chunks): small first so PE starts early. All data
    # batches issue serially on the sync queue (strict arrival order); only
    # the tiny aux transfers ride the scalar DGE.
    ABATCH = ([(0, 2), (2, 6), (6, 14), (14, 22)]
              + [(k, k + 12) for k in range(22, 82, 12)]
              + [(82, 90), (90, 96)])
    BBATCH = ([(k, k + 12) for k in range(0, 84, 12)]
              + [(84, 92), (92, 96)])

    with tile.TileContext(nc) as tc:
        with (
            tc.tile_pool(name="g", bufs=1) as g,
            tc.tile_pool(name="sp", bufs=4) as sp,
            tc.tile_pool(name="ps", bufs=1, space="PSUM") as psp,
        ):
            tA = g.tile([BLK, KCH, 640], FP8)
            tB = g.tile([BLK, KCH, 640], FP8)
            auxf_t = g.tile([BLK, 132], F32)
            auxb_t = g.tile([1, 1408], BF16)
            outt = g.tile([BLK, 92], F32)
            acc = outt[:, 0:6]
            pstats = outt[:, 6:48]
            estats = outt[:, 48:90]
            acc2 = outt[:, 90:92]  # ACT-accumulated E for B0h, B2
            tri_t = auxf_t[:, 0:128]
            bias0 = auxf_t[:, 128:129]
            bias1 = auxf_t[:, 129:130]
            biash0 = auxf_t[:, 130:131]
            biash1 = auxf_t[:, 131:132]
            vq = auxb_t  # cols 0:1280 = v_q (union order), 1280:1408 = ones
            ones = auxb_t[:, 1280:1408]

            # PSUM tiles
            tA0 = psp.tile([BLK, 512], F32, tag="tA0")  # s0 x union[0:512)
            tA1 = psp.tile([BLK, 128], F32, tag="tA1")  # s0 x union[512:640)
            tA2 = psp.tile([BLK, 512], F32, tag="tA2")  # s1 x union[128:640)
            tB0 = psp.tile([BLK, 512], F32, tag="tB0")  # s0 x union[640:1152)
            tB1 = psp.tile([BLK, 512], F32, tag="tB1")  # s1 x union[640:1152)
            tB2 = psp.tile([BLK, 128], F32, tag="tB2")  # s1 x union[1152:1280)

            # ---- DMA issue: first data batch foremost; aux transfers ride
            # the scalar (Activation) DGE so they don't delay the data queue.
            nc.sync.dma_start(tA[:, 0:2, :], xA[:, 0:2, :])
            nc.scalar.dma_start(auxb_t[:], auxb[:])
            nc.scalar.dma_start(auxf_t[:], auxf[:])
            for (k0, k1) in ABATCH[1:]:
                nc.sync.dma_start(tA[:, k0:k1, :], xA[:, k0:k1, :])
            for (k0, k1) in BBATCH:
                nc.sync.dma_start(tB[:, k0:k1, :], xB[:, k0:k1, :])

            # preload Exp activation table (off critical path)
            zcol = g.tile([BLK, 1], F32)
            nc.gpsimd.memset(zcol[:], 0.0)
            zscr = g.tile([BLK, 1], F32)
            nc.scalar.activation(zscr[:], zcol[:], EXP)

            # PE prewarm: trigger the power-ramp during the DMA-limited start
            # so the first real matmuls run at full clock. Ends before batch0
            # lands, so it never delays real work.
            warm = g.tile([BLK, 2, BLK], FP8)
            nc.gpsimd.memset(warm[:], 0.0)
            wps = psp.tile([BLK, BLK], F32, tag="wps")
            for _ in range(14):
                nc.tensor.matmul(wps[:], warm[:, :, 0:128], warm[:],
                                 start=True, stop=True, perf_mode=DR,
                                 skip_group_check=True)

            def a_step(k, last=False):
                lhs0 = tA[:, k : k + 2, 0:128]
                lhs1 = tA[:, k : k + 2, 128:256]
                nc.tensor.matmul(tA0[:], lhs0, tA[:, k : k + 2, 0:512],
                                 start=False, stop=last, perf_mode=DR)
                nc.tensor.matmul(tA1[:], lhs0, tA[:, k : k + 2, 512:640],
                                 start=False, stop=last, perf_mode=DR)
                nc.tensor.matmul(tA2[:], lhs1, tA[:, k : k + 2, 128:640],
                                 start=False, stop=last, perf_mode=DR)

            # ALL v-folds first (start=True, initializing each accumulation
            # group) — they ride the DMA ramp right after the prewarm: the PE
            # stream stays pure-DR afterwards and post-B is unblocked the
            # moment the last DR matmul retires.
            nc.tensor.matmul(tA0[:], ones, vq[:, 0:512], start=True, stop=False)
            nc.tensor.matmul(tA1[:], ones, vq[:, 512:640], start=True, stop=False)
            nc.tensor.matmul(tA2[:], ones, vq[:, 128:640], start=True, stop=False)
            nc.tensor.matmul(tB0[:], ones, vq[:, 640:1152], start=True, stop=False)
            nc.tensor.matmul(tB1[:], ones, vq[:, 640:1152], start=True, stop=False)
            nc.tensor.matmul(tB2[:], ones, vq[:, 1152:1280], start=True, stop=False)

            # ---- phase A matmuls (tA stays resident) ----
            for k in range(0, KCH, 2):
                a_step(k, last=(k == KCH - 2))

            # ---- post processing helpers ----
            # ACT: exp (PSUM read); DVE: bn_stats on PSUM (sum P, sum P^2 per
            # row) and on the exp output (sum E per row). Weights applied on
            # the host from the per-region 6-tuples.
            def post_full(pm, w, bias, r, e_acc=None):
                scr = sp.tile([BLK, w], F32, tag="scr")
                nc.scalar.activation(scr[:], pm, EXP, bias=bias, scale=S2E)
                if e_acc is None:
                    nc.vector.bn_stats(estats[:, 6 * r : 6 * r + 6], scr[:])
                else:
                    nc.vector.tensor_reduce(acc2[:, e_acc : e_acc + 1], scr[:],
                                            mybir.AxisListType.X, ADD)
                nc.vector.bn_stats(pstats[:, 6 * r : 6 * r + 6], pm)

            def post_diag(pd, bias, e_i, s1_i, s2_i):
                et = sp.tile([BLK, BLK], F32, tag="et")
                nc.scalar.activation(et[:], pd, EXP, bias=bias, scale=S2E)
                me = sp.tile([BLK, BLK], F32, tag="me")
                nc.vector.scalar_tensor_tensor(
                    out=me[:], in0=et[:], scalar=1.0, in1=tri_t, op0=MULT, op1=MULT,
                    accum_out=acc[:, e_i : e_i + 1])
                mu = sp.tile([BLK, BLK], F32, tag="mu")
                nc.vector.scalar_tensor_tensor(
                    out=mu[:], in0=pd, scalar=1.0, in1=tri_t, op0=MULT, op1=MULT,
                    accum_out=acc[:, s1_i : s1_i + 1])
                ms2 = sp.tile([BLK, BLK], F32, tag="ms2")
                nc.vector.scalar_tensor_tensor(
                    out=ms2[:], in0=mu[:], scalar=1.0, in1=mu[:], op0=MULT, op1=MULT,
                    accum_out=acc[:, s2_i : s2_i + 1])

            # ---- post A (overlaps phase B matmuls) ----
            # P-stat/E-stat region order: 0=A0f 1=A1 2=A2f 3=B1 4=B0f 5=B0h 6=B2
            post_diag(tA0[:, 0:128], bias0, 0, 2, 4)
            post_full(tA0[:, 128:512], 384, bias0, 0)
            post_full(tA1[:], 128, bias0, 1)
            post_diag(tA2[:, 0:128], bias1, 1, 3, 5)
            post_full(tA2[:, 128:512], 384, bias1, 2)

            # ---- phase B matmuls (tB resident, DMA'd upfront) ----
            for k in range(0, KCH, 2):
                last = (k == KCH - 2)
                lhs0 = tA[:, k : k + 2, 0:128]
                lhs1 = tA[:, k : k + 2, 128:256]
                nc.tensor.matmul(tB0[:], lhs0, tB[:, k : k + 2, 0:512],
                                 start=False, stop=last, perf_mode=DR)
                nc.tensor.matmul(tB1[:], lhs1, tB[:, k : k + 2, 0:512],
                                 start=False, stop=last, perf_mode=DR)
                nc.tensor.matmul(tB2[:], lhs1, tB[:, k : k + 2, 512:640],
                                 start=False, stop=last, perf_mode=DR)

            # ---- post B (serial tail; largest region first, smallest last) ----
            post_full(tB1[:], 512, bias1, 3)
            post_full(tB0[:, 0:384], 384, bias0, 4)
            post_full(tB0[:, 384:512], 128, bias0, 5, e_acc=0)
            post_full(tB2[:], 128, bias1, 6, e_acc=1)

            nc.sync.dma_start(out_stats[:], outt[:])
    nc.compile()
    return nc


def _get(name, builder):
    if name not in _cache:
        _cache[name] = builder()
    return _cache[name]


def _run(nc, in_maps, tag):
    if _trace_enabled():
        try:
            import profhook

            profhook.install()
        except Exception:
            pass
        import tempfile

        res = run_bass_kernel_spmd(
            nc, in_maps, list(range(NC_N)), trace=True,
            tmpdir=tempfile.mkdtemp(prefix=f"ktrace_{tag}_"),
        )
        KERNEL_EXEC_NS.append((tag, res.exec_time_ns))
        return res.results
    return run_bass_kernel_spmd(nc, in_maps, list(range(NC_N))).results


def kernel(features):
    x = np.asarray(features).reshape(B, D)
    xq8 = x.astype(NP_FP8)

    # host-side prep (f64 exact)
    xqf = xq8.astype(np.float32)
    sq = (xqf.astype(np.float64) ** 2).sum(1)
    Mbar = sq.mean()
    a = sq + Mbar - CC
    vprime = (Mbar - sq) / 2.0
    vq16 = vprime.astype(NP_BF16)
    vqf = vq16.astype(np.float64)
    delta = vprime - vqf

    xT_full = np.ascontiguousarray(xq8.T)  # (D, B)
    tri = np.triu(np.ones((BLK, BLK), np.float32), k=1)
    in_maps = []
    col_list = []
    for c in range(NC_N):
        cols = (256 * c + np.arange(UNION)) % B
        col_list.append(cols)
        xu = xT_full[:, cols].reshape(KCH, BLK, UNION).transpose(1, 0, 2)
        rows0 = np.arange(256 * c, 256 * c + 128)
        rows1 = rows0 + 128
        auxf = np.empty((BLK, 132), np.float32)
        auxf[:, 0:128] = tri
        auxf[:, 128] = (-SS * a[rows0]).astype(np.float32)
        auxf[:, 129] = (-SS * a[rows1]).astype(np.float32)
        auxf[:, 130] = auxf[:, 128] + LN_HALF
        auxf[:, 131] = auxf[:, 129] + LN_HALF
        auxb = np.empty((1, 1408), NP_BF16)
        auxb[0, 0:1280] = vq16[cols]
        auxb[0, 1280:1408] = NP_BF16(1.0)
        in_maps.append({
            "xA": np.ascontiguousarray(xu[:, :, 0:640]),
            "xB": np.ascontiguousarray(xu[:, :, 640:1280]),
            "auxf": auxf,
            "auxb": auxb,
        })

    nc = _get("main", _build_kernel)
    res = _run(nc, in_maps, "main")

    # ---- host combine (f64) ----
    # out cols: 0:6 diag acc [E_d0, E_d1, S1_d0, S1_d1, S2_d0, S2_d1],
    # 6:48 P bn_stats (7 regions x [c,m,M2] even/odd), 48:90 E bn_stats.
    # Regions: 0=A0f(s0,1) 1=A1(s0,1) 2=A2f(s1,1) 3=B1(s1,1) 4=B0f(s0,1)
    #          5=B0h(s0,.5) 6=B2(s1,.5)
    E_tot = 0.0
    S1_tot = 0.0
    S2_tot = 0.0
    m_i = 1087.0 - np.arange(128)
    n_mult = np.full(1152, 128.0)
    n_mult[0:128] = np.arange(128)
    n_mult[1024:1152] = 64.0
    R_WT = [1.0, 1.0, 1.0, 1.0, 1.0, 0.5, 0.5]
    R_STRIP = [0, 0, 1, 1, 0, 0, 1]
    for c in range(NC_N):
        o = res[c]["out_stats"].astype(np.float64)
        cols = col_list[c]
        rows0 = np.arange(256 * c, 256 * c + 128)
        rows1 = rows0 + 128

        def tup(base, r):
            b = base + 6 * r
            ce, me_, M2e = o[:, b], o[:, b + 1], o[:, b + 2]
            co, mo, M2o = o[:, b + 3], o[:, b + 4], o[:, b + 5]
            rsum = ce * me_ + co * mo
            rsum2 = M2e + ce * me_ * me_ + M2o + co * mo * mo
            return rsum, rsum2

        R0 = o[:, 2].copy()
        R1 = o[:, 3].copy()
        P2 = o[:, 4].sum() + o[:, 5].sum()
        E_tot += o[:, 0].sum() + o[:, 1].sum()
        for r in range(7):
            rp, rp2 = tup(6, r)
            w = R_WT[r]
            if r < 5:
                re_, _ = tup(48, r)
                E_tot += w * re_.sum()
            else:
                E_tot += w * o[:, 90 + (r - 5)].sum()
            if R_STRIP[r] == 0:
                R0 += w * rp
            else:
                R1 += w * rp
            P2 += w * rp2.sum()
        d_u0 = delta[cols[0:1152]]
        d_u1 = delta[cols[128:1280]]
        S1_c = (-2.0 * (R0.sum() + R1.sum())
                + (m_i * (a[rows0] + a[rows1])).sum()
                - 2.0 * ((n_mult * d_u0).sum() + (n_mult * d_u1).sum()))
        mid = (a[rows0] * R0).sum() + (a[rows1] * R1).sum()
        cum0 = np.cumsum(d_u0[0:128][::-1])[::-1]
        rowd0 = (np.concatenate([cum0[1:], [0.0]])
                 + d_u0[128:1024].sum() + 0.5 * d_u0[1024:1152].sum())
        cum1 = np.cumsum(d_u1[0:128][::-1])[::-1]
        rowd1 = (np.concatenate([cum1[1:], [0.0]])
                 + d_u1[128:1024].sum() + 0.5 * d_u1[1024:1152].sum())
        C2 = ((m_i * (a[rows0] ** 2 + a[rows1] ** 2)).sum()
              - 4.0 * ((a[rows0] * rowd0).sum() + (a[rows1] * rowd1).sum())
              + 4.0 * ((n_mult * d_u0 ** 2).sum() + (n_mult * d_u1 ** 2).sum()))
        S1_tot += S1_c
        S2_tot += 4.0 * P2 - 4.0 * mid + C2

    N = float(N_PAIRS)
    mean_u = S1_tot / N
    mean = (mean_u + CC) / D
    var_u = (S2_tot - N * mean_u * mean_u) / (N - 1.0)
    std = np.sqrt(var_u) / D
    loss = CC * SS - np.log(E_tot) + np.log(N)
    feat_norm = np.sqrt((x.astype(np.float64) ** 2).sum(1)).mean()

    return (
        np.float32(loss),
        np.float32(feat_norm),
        np.float32(mean),
        np.float32(std),
    )


if __name__ == "__main__":
    f = np.random.default_rng(0).standard_normal((B, 16, 768), dtype=np.float32)
    print(kernel(features=f))
